# revision 6
# baseline (speedup 1.0000x reference)
"""Expert-LoRA routed delta kernel for Trainium2 (8 NeuronCores).

Math (per batch b, with routing resolved on host):
    out[b] = base[b] + x[b] @ At_b @ Bwt_b
where
    At_b  [H, 32] = concat_k A_{e_k}^T              (e_k = top_k_indices[b, k])
    Bwt_b [32, H] = concat_k (w_{b,k} * scaling * B_{e_k}^T)

Host-side prep folds everything cheap into input layout:
  * expert gather + gate weights + lora scaling -> tiny At/Bwt tables;
  * x is pre-transposed to an h-major tiled layout (xt[half, p, j, s]) so the
    tensor engine can contract over H without any on-chip transposes, and
    each DMA reads one fully contiguous block.

The kernel is HBM-bandwidth-bound (~358 GB/s per NeuronCore), so all bulk
I/O is compressed (rel-err budget 2e-2; measured total err ~1.1e-2 worst):
  * x in bf16 (14.7 MB),
  * base in fp8, pre-divided by a host-calibrated output scale (7.3 MB),
  * out in int8 in the scaled domain (7.3 MB), dequantized on host.
The scale folds into Bwt (bwt_s = Bwt/scale) so the device program is
scale-free. Per-core traffic: 88 MB -> 29.4 MB.

Device pipeline per core (= one batch; B == n_cores == 8):
  for each 512-row S-macro: load xT halves -> 28 accumulating matmuls
  (rank-32 down-projection, N=512) -> per 128-row block: 7 up-projection
  matmuls (K=32, N=512) grouped into 1536-wide PSUM chunks + wide vector
  add with base (wide chunks amortize the DVE ~151-cycle/op overhead)
  -> store. DVE ~65us, PE ~48us, DMA ~82us -> DMA-bound.

Sharding: data-parallel over batch (spec sharding_hint), SPMD program.
"""

import sys

if "/opt/trn_rl_repo" not in sys.path:
    sys.path.insert(0, "/opt/trn_rl_repo")

import numpy as np

# Problem shape (hardcoded per contract; must match setup_inputs()).
B, S, H = 8, 2048, 3584
E, R, TOPK = 8, 16, 2
KR = TOPK * R  # 32 = concatenated rank
SCALING = 32.0 / 16.0
N_CORES = 8

S_BLK = 128
NS = S // S_BLK  # 16 s-blocks
HB = H // 128  # 28 h-blocks of 128
HC = H // 512  # 7 h-chunks of 512
NMAC = S // 512  # 4 S-macros of 512 rows
HHALF = HB // 2  # 14 h-blocks per xT half-tile
CHUNK_PLAN = [(0, 1536), (1536, 1536), (3072, 512)]  # up-proj (h_off, width)

# I/O compression config.
XT_DT = "bfloat16"
TAB_DT = "bfloat16"  # at / bwt tables (tiny but must match matmul operand)
OUT_MODE = "i8"  # "i8": scaled int8 out + fp8 scaled base | "bf16"
SCALE_MARGIN = 1.30  # quantization headroom over sampled absmax

_CACHE: dict = {}


def _dtypes():
    import concourse.mybir as mybir

    if OUT_MODE == "i8":
        # base is pre-divided by scale (values ~<0.2): e4m3 keeps them in
        # normal range; even flushed subnormals would be harmless.
        return getattr(mybir.dt, XT_DT), getattr(mybir.dt, TAB_DT), \
            mybir.dt.float8e4, mybir.dt.int8
    return getattr(mybir.dt, XT_DT), getattr(mybir.dt, TAB_DT), \
        mybir.dt.float8e3, mybir.dt.bfloat16


def _split_sync_waits(nc, max_waits=1):
    """This walrus build rejects >max_waits sync-wait commands on a single
    instruction (setupSyncWait: 'Too many sync wait commands'). Hoist excess
    waits onto same-engine NOPs inserted immediately before the instruction.
    Same-queue ordering makes this equivalent: the engine blocks on each
    hoisted wait before reaching the original instruction. Monotonic (ge)
    waits are hoisted first; eq-waits stay on the instruction when possible.
    """
    import concourse.mybir as mybir

    for fn in nc.m.functions:
        for bb in fn.blocks:
            new_insts = []
            for inst in bb.instructions:
                si = inst.sync_info
                if si is not None and si.on_wait and len(si.on_wait) > max_waits:
                    waits = list(si.on_wait)
                    ge = [w for w in waits if w.wait_mode != "sem-eq-imm"]
                    eq = [w for w in waits if w.wait_mode == "sem-eq-imm"]
                    keep = (eq + ge)[-max_waits:]
                    hoist = (eq + ge)[:-max_waits]
                    for w in hoist:
                        new_insts.append(
                            mybir.InstNoOp(
                                name=f"I-{nc.next_id()}",
                                engine=inst.engine,
                                bass_nofuse=True,
                                sync_info=mybir.SyncInfo(on_wait=[w], on_update=[]),
                            )
                        )
                    inst.sync_info = mybir.SyncInfo(
                        on_wait=keep, on_update=list(si.on_update or [])
                    )
                new_insts.append(inst)
            bb.instructions[:] = new_insts


def build_nc(reps=1, dma_only=False, io_bufs=2, xt_bufs=4,
             store_on_act=True, base_eng="sync"):
    """Build the single-core Bass program (SPMD: same program on all cores).

    reps>1 repeats the whole pipeline (same I/O, idempotent) — used only for
    slope-based device-time measurement in test.py. dma_only strips compute
    (out <- cast(base), xT still loaded) to calibrate the pure DMA roofline.
    """
    import concourse.bass as bass
    import concourse.mybir as mybir
    import concourse.tile as tile

    f32 = mybir.dt.float32
    xt_dt, tab_dt, base_dt, out_dt = _dtypes()

    nc = bass.Bass()
    # xt[half, p, j, s] = x[(half//2)*512 + s, (half%2)*14*128 + j*128 + p]
    # (partition-major so each half loads as one fully contiguous DMA)
    xt = nc.dram_tensor("xt", [2 * NMAC, 128, HHALF, 512], xt_dt, kind="ExternalInput")
    base = nc.dram_tensor("base", [S, H], base_dt, kind="ExternalInput")
    # at[p, j, r] = A_cat^T[j*128 + p, r] (pre-striped on host)
    at = nc.dram_tensor("at", [128, HB, KR], tab_dt, kind="ExternalInput")
    bwt = nc.dram_tensor("bwt", [KR, H], tab_dt, kind="ExternalInput")
    out = nc.dram_tensor("out", [S, H], out_dt, kind="ExternalOutput")

    # Loads go on the SP HWDGE ring; stores on the ACT ring so a store
    # waiting for compute never head-of-line-blocks the next loads.
    store_eng = nc.scalar if store_on_act else nc.sync
    b_eng = {"sync": nc.sync, "scalar": nc.scalar, "gpsimd": nc.gpsimd}[base_eng]

    with tile.TileContext(nc) as tc:
        with (
            tc.tile_pool(name="const", bufs=1) as const_pool,
            tc.tile_pool(name="xth", bufs=xt_bufs) as xt_pool,
            tc.tile_pool(name="bin", bufs=io_bufs) as b_pool,
            tc.tile_pool(name="oout", bufs=io_bufs) as o_pool,
            tc.tile_pool(name="low", bufs=3) as low_pool,
            tc.tile_pool(name="plow", bufs=2, space="PSUM") as plow_pool,
            tc.tile_pool(name="pd", bufs=2, space="PSUM") as pd_pool,
        ):
            at_sb = const_pool.tile([128, HB, KR], tab_dt)
            nc.sync.dma_start(at_sb[:], at[:])
            bwt_sb = const_pool.tile([KR, H], tab_dt)
            nc.sync.dma_start(bwt_sb[:], bwt[:])

            for m in range(NMAC * reps):
                m = m % NMAC
                # xT halves: [128 h-partitions, 14 h-blocks, 512 s]
                halves = []
                for hf in range(2):
                    xh = xt_pool.tile([128, HHALF, 512], xt_dt, tag="xth")
                    nc.sync.dma_start(xh[:], xt[2 * m + hf])
                    halves.append(xh)

                if not dma_only:
                    # down-projection: lowT[kr, s] = sum_h At[h, kr] * xT[h, s]
                    plow = plow_pool.tile([KR, 512], f32, tag="plow")
                    for j in range(HB):
                        nc.tensor.matmul(
                            plow[:],
                            at_sb[:, j, :],
                            halves[j // HHALF][:, j % HHALF, :],
                            start=(j == 0),
                            stop=(j == HB - 1),
                        )
                    lowT = low_pool.tile([KR, 512], tab_dt, tag="lowT")
                    nc.vector.tensor_copy(lowT[:], plow[:])

                for g in range(4):  # 128-row s-blocks within the macro
                    srow = m * 512 + g * S_BLK
                    bt = b_pool.tile([S_BLK, H], base_dt, tag="base")
                    b_eng.dma_start(bt[:], base[srow : srow + S_BLK, :])
                    ot = o_pool.tile([S_BLK, H], out_dt, tag="out")
                    if dma_only:
                        nc.vector.tensor_copy(ot[:], bt[:])
                        store_eng.dma_start(out[srow : srow + S_BLK, :], ot[:])
                        continue
                    # up-projection (K=32) into wide PSUM chunks + base add
                    for off, width in CHUNK_PLAN:
                        pd = pd_pool.tile([S_BLK, 1536], f32, tag="pd")
                        for i in range(width // 512):
                            nc.tensor.matmul(
                                pd[:, i * 512 : (i + 1) * 512],
                                lowT[:, g * S_BLK : (g + 1) * S_BLK],
                                bwt_sb[:, off + i * 512 : off + (i + 1) * 512],
                                start=True,
                                stop=True,
                            )
                        nc.vector.tensor_add(
                            ot[:, off : off + width],
                            pd[:, :width],
                            bt[:, off : off + width],
                        )
                    store_eng.dma_start(out[srow : srow + S_BLK, :], ot[:])

    _split_sync_waits(nc)
    return nc


def make_in_maps(x, base_output, lora_A, lora_B, top_k_weights, top_k_indices):
    """Host-side prep: expert gather, gate/scaling fold, x h-major relayout,
    compression to device I/O dtypes, and (i8 mode) output-scale calibration.
    Stores the dequantization scale in _CACHE['out_scale']."""
    import concourse.mybir as mybir

    xt_dt, tab_dt, base_dt, out_dt = _dtypes()
    np_xt = mybir.dt.np(xt_dt)
    np_tab = mybir.dt.np(tab_dt)
    np_base = mybir.dt.np(base_dt)

    x = np.asarray(x, dtype=np.float32)
    base_output = np.asarray(base_output, dtype=np.float32)
    lora_A = np.asarray(lora_A, dtype=np.float32)
    lora_B = np.asarray(lora_B, dtype=np.float32)
    w = np.asarray(top_k_weights, dtype=np.float32)
    idx = np.asarray(top_k_indices)

    A_sel = lora_A[idx]  # [B, K, R, H]
    At = A_sel.reshape(B, KR, H).transpose(0, 2, 1)  # [B, H, 32]
    # stripe h-major: At_dev[b, p, j, r] = At[b, j*128 + p, r]
    At_dev = np.ascontiguousarray(
        At.reshape(B, HB, 128, KR).transpose(0, 2, 1, 3)
    ).astype(np_tab)  # [B, 128, 28, 32]
    B_sel = lora_B[idx]  # [B, K, H, R]
    Bw = B_sel * (w * SCALING)[:, :, None, None]
    Bwt = np.ascontiguousarray(
        Bw.transpose(0, 1, 3, 2).reshape(B, KR, H)
    )  # [B, 32, H]

    if OUT_MODE == "i8":
        # Calibrate the int8 output scale from an exact delta on a 1/64 row
        # sample; absmax of the full tensor exceeds the sample absmax only
        # by the Gaussian-extreme ratio (~1.15x), covered by SCALE_MARGIN.
        xs = x[:, ::64].astype(np.float32)  # [B, 32, H]
        low_s = np.einsum("bsh,bhr->bsr", xs, At)
        delta_s = np.einsum("bsr,brh->bsh", low_s, Bwt)
        samp_max = float(np.abs(delta_s + base_output[:, ::64]).max())
        scale = samp_max * SCALE_MARGIN / 127.0
    else:
        scale = 1.0
    _CACHE["out_scale"] = scale

    Bwt_dev = (Bwt / scale).astype(np_tab)
    base_dev = (base_output / scale).astype(np_base)

    # x -> xt[half, p, j, s]: partition-major tiles, each half fully
    # contiguous per SBUF partition line
    # xt[b, 2m+hf, p, j, s] = x[b, m*512 + s, hf*1792 + j*128 + p]
    xt = np.ascontiguousarray(
        x.astype(np_xt)
        .reshape(B, NMAC, 512, 2, HHALF, 128)
        .transpose(0, 1, 3, 5, 4, 2)  # [B, m, hf, p, j, s]
        .reshape(B, 2 * NMAC, 128, HHALF, 512)
    )

    return [
        {
            "xt": xt[b],
            "base": np.ascontiguousarray(base_dev[b]),
            "at": At_dev[b],
            "bwt": Bwt_dev[b],
        }
        for b in range(B)
    ]


def kernel(x, base_output, lora_A, lora_B, top_k_weights, top_k_indices):
    from concourse.bass_utils import run_bass_kernel_spmd

    nc = _CACHE.get("nc")
    if nc is None:
        nc = build_nc()
        _CACHE["nc"] = nc

    in_maps = make_in_maps(
        x, base_output, lora_A, lora_B, top_k_weights, top_k_indices
    )
    scale = _CACHE["out_scale"]
    res = run_bass_kernel_spmd(nc, in_maps, list(range(N_CORES)))
    out = np.stack(
        [np.asarray(res.results[b]["out"]) for b in range(B)], axis=0
    ).astype(np.float32)
    if scale != 1.0:
        out *= scale
    return out


# revision 10
# speedup vs baseline: 1.1356x; 1.1356x over previous
"""Expert-LoRA routed delta kernel for Trainium2 (8 NeuronCores).

Math (per batch b, with routing resolved on host):
    out[b] = base[b] + x[b] @ At_b @ Bwt_b
where
    At_b  [H, 32] = concat_k A_{e_k}^T              (e_k = top_k_indices[b, k])
    Bwt_b [32, H] = concat_k (w_{b,k} * scaling * B_{e_k}^T)

Host-side prep folds everything cheap into input layout:
  * expert gather + gate weights + lora scaling -> tiny At/Bwt tables;
  * x is pre-transposed to an h-major tiled layout (xt[half, p, j, s]) so the
    tensor engine can contract over H without any on-chip transposes, and
    each DMA reads one fully contiguous block.

The kernel is HBM-bandwidth-bound (~358 GB/s per NeuronCore), so all bulk
I/O is compressed (rel-err budget 2e-2; measured total err ~1.1e-2 worst):
  * x in bf16 (14.7 MB),
  * base in fp8, pre-divided by a host-calibrated output scale (7.3 MB),
  * out in int8 in the scaled domain (7.3 MB), dequantized on host.
The scale folds into Bwt (bwt_s = Bwt/scale) so the device program is
scale-free. Per-core traffic: 88 MB -> 29.4 MB.

Device pipeline per core (= one batch; B == n_cores == 8):
  for each 512-row S-macro: load xT halves -> 28 accumulating matmuls
  (rank-32 down-projection, N=512) -> per 128-row block: 7 up-projection
  matmuls (K=32, N=512) grouped into 1536-wide PSUM chunks + wide vector
  add with base (wide chunks amortize the DVE ~151-cycle/op overhead)
  -> store. DVE ~65us, PE ~48us, DMA ~82us -> DMA-bound.

Sharding: data-parallel over batch (spec sharding_hint), SPMD program.
"""

import sys

if "/opt/trn_rl_repo" not in sys.path:
    sys.path.insert(0, "/opt/trn_rl_repo")

import numpy as np

# Problem shape (hardcoded per contract; must match setup_inputs()).
B, S, H = 8, 2048, 3584
E, R, TOPK = 8, 16, 2
KR = TOPK * R  # 32 = concatenated rank
SCALING = 32.0 / 16.0
N_CORES = 8

S_BLK = 128
NS = S // S_BLK  # 16 s-blocks
HB = H // 128  # 28 h-blocks of 128
HC = H // 512  # 7 h-chunks of 512
NMAC = S // 512  # 4 S-macros of 512 rows
HHALF = HB // 2  # 14 h-blocks per xT half-tile
CHUNK_PLAN = [(0, 1536), (1536, 1536), (3072, 512)]  # up-proj (h_off, width)

# I/O compression config.
XT_DT = "bfloat16"
TAB_DT = "bfloat16"  # at / bwt tables (tiny but must match matmul operand)
OUT_MODE = "i8"  # "i8": scaled int8 out + fp8 scaled base | "bf16"
SCALE_MARGIN = 1.30  # quantization headroom over sampled absmax

_CACHE: dict = {}


def _dtypes():
    import concourse.mybir as mybir

    if OUT_MODE == "i8":
        # base is pre-divided by scale (values ~<0.2): e4m3 keeps them in
        # normal range; even flushed subnormals would be harmless.
        return getattr(mybir.dt, XT_DT), getattr(mybir.dt, TAB_DT), \
            mybir.dt.float8e4, mybir.dt.int8
    return getattr(mybir.dt, XT_DT), getattr(mybir.dt, TAB_DT), \
        mybir.dt.float8e3, mybir.dt.bfloat16


def _split_sync_waits(nc, max_waits=1):
    """This walrus build rejects >max_waits sync-wait commands on a single
    instruction (setupSyncWait: 'Too many sync wait commands'). Hoist excess
    waits onto same-engine NOPs inserted immediately before the instruction.
    Same-queue ordering makes this equivalent: the engine blocks on each
    hoisted wait before reaching the original instruction. Monotonic (ge)
    waits are hoisted first; eq-waits stay on the instruction when possible.
    """
    import concourse.mybir as mybir

    for fn in nc.m.functions:
        for bb in fn.blocks:
            new_insts = []
            for inst in bb.instructions:
                si = inst.sync_info
                if si is not None and si.on_wait and len(si.on_wait) > max_waits:
                    waits = list(si.on_wait)
                    ge = [w for w in waits if w.wait_mode != "sem-eq-imm"]
                    eq = [w for w in waits if w.wait_mode == "sem-eq-imm"]
                    keep = (eq + ge)[-max_waits:]
                    hoist = (eq + ge)[:-max_waits]
                    for w in hoist:
                        new_insts.append(
                            mybir.InstNoOp(
                                name=f"I-{nc.next_id()}",
                                engine=inst.engine,
                                bass_nofuse=True,
                                sync_info=mybir.SyncInfo(on_wait=[w], on_update=[]),
                            )
                        )
                    inst.sync_info = mybir.SyncInfo(
                        on_wait=keep, on_update=list(si.on_update or [])
                    )
                new_insts.append(inst)
            bb.instructions[:] = new_insts


def build_nc(reps=1, dma_only=False, io_bufs=2, xt_bufs=4,
             store_on_act=True, base_eng="sync", batch_io=True):
    """Build the single-core Bass program (SPMD: same program on all cores).

    reps>1 repeats the whole pipeline (same I/O, idempotent) — used only for
    slope-based device-time measurement in test.py. dma_only strips compute
    (out <- cast(base), xT still loaded) to calibrate the pure DMA roofline.
    """
    import concourse.bass as bass
    import concourse.mybir as mybir
    import concourse.tile as tile

    f32 = mybir.dt.float32
    xt_dt, tab_dt, base_dt, out_dt = _dtypes()

    nc = bass.Bass()
    # xt[half, p, j, s] = x[(half//2)*512 + s, (half%2)*14*128 + j*128 + p]
    # (partition-major so each half loads as one fully contiguous DMA)
    xt = nc.dram_tensor("xt", [2 * NMAC, 128, HHALF, 512], xt_dt, kind="ExternalInput")
    base = nc.dram_tensor("base", [S, H], base_dt, kind="ExternalInput")
    # at[p, j, r] = A_cat^T[j*128 + p, r] (pre-striped on host)
    at = nc.dram_tensor("at", [128, HB, KR], tab_dt, kind="ExternalInput")
    bwt = nc.dram_tensor("bwt", [KR, H], tab_dt, kind="ExternalInput")
    out = nc.dram_tensor("out", [S, H], out_dt, kind="ExternalOutput")

    # Loads go on the SP HWDGE ring; stores on the ACT ring so a store
    # waiting for compute never head-of-line-blocks the next loads.
    store_eng = nc.scalar if store_on_act else nc.sync
    b_eng = {"sync": nc.sync, "scalar": nc.scalar, "gpsimd": nc.gpsimd}[base_eng]

    with tile.TileContext(nc) as tc:
        with (
            tc.tile_pool(name="const", bufs=1) as const_pool,
            tc.tile_pool(name="xth", bufs=xt_bufs) as xt_pool,
            tc.tile_pool(name="bin", bufs=io_bufs) as b_pool,
            tc.tile_pool(name="oout", bufs=io_bufs) as o_pool,
            tc.tile_pool(name="low", bufs=3) as low_pool,
            tc.tile_pool(name="plow", bufs=2, space="PSUM") as plow_pool,
            tc.tile_pool(name="pd", bufs=2, space="PSUM") as pd_pool,
        ):
            # Const tables ride the (idle-at-start) store ring so they land
            # concurrently with the first xT load on the SP ring.
            at_sb = const_pool.tile([128, HB, KR], tab_dt)
            store_eng.dma_start(at_sb[:], at[:])
            bwt_sb = const_pool.tile([KR, H], tab_dt)
            store_eng.dma_start(bwt_sb[:], bwt[:])

            for m in range(NMAC * reps):
                m = m % NMAC
                # xT halves: [128 h-partitions, 14 h-blocks, 512 s]
                halves = []
                for hf in range(2):
                    xh = xt_pool.tile([128, HHALF, 512], xt_dt, tag="xth")
                    nc.sync.dma_start(xh[:], xt[2 * m + hf])
                    halves.append(xh)

                if not dma_only:
                    # down-projection: lowT[kr, s] = sum_h At[h, kr] * xT[h, s]
                    plow = plow_pool.tile([KR, 512], f32, tag="plow")
                    for j in range(HB):
                        nc.tensor.matmul(
                            plow[:],
                            at_sb[:, j, :],
                            halves[j // HHALF][:, j % HHALF, :],
                            start=(j == 0),
                            stop=(j == HB - 1),
                        )
                    lowT = low_pool.tile([KR, 512], tab_dt, tag="lowT")
                    nc.vector.tensor_copy(lowT[:], plow[:])

                if batch_io:
                    # whole-macro base load / out store: one 512-row DMA
                    # each, laid out [p, k, h] (k = s-block within macro)
                    mrows = slice(m * 512, (m + 1) * 512)
                    bt4 = b_pool.tile([S_BLK, 4, H], base_dt, tag="base")
                    b_eng.dma_start(
                        bt4[:], base[mrows].rearrange("(k p) h -> p k h", p=128)
                    )
                    ot4 = o_pool.tile([S_BLK, 4, H], out_dt, tag="out")
                    for g in range(4):
                        if dma_only:
                            nc.vector.tensor_copy(ot4[:, g, :], bt4[:, g, :])
                            continue
                        for off, width in CHUNK_PLAN:
                            pd = pd_pool.tile([S_BLK, 1536], f32, tag="pd")
                            for i in range(width // 512):
                                nc.tensor.matmul(
                                    pd[:, i * 512 : (i + 1) * 512],
                                    lowT[:, g * S_BLK : (g + 1) * S_BLK],
                                    bwt_sb[:, off + i * 512 : off + (i + 1) * 512],
                                    start=True,
                                    stop=True,
                                )
                            nc.vector.tensor_add(
                                ot4[:, g, off : off + width],
                                pd[:, :width],
                                bt4[:, g, off : off + width],
                            )
                    store_eng.dma_start(
                        out[mrows].rearrange("(k p) h -> p k h", p=128), ot4[:]
                    )
                    continue

                for g in range(4):  # 128-row s-blocks within the macro
                    srow = m * 512 + g * S_BLK
                    bt = b_pool.tile([S_BLK, H], base_dt, tag="base")
                    b_eng.dma_start(bt[:], base[srow : srow + S_BLK, :])
                    ot = o_pool.tile([S_BLK, H], out_dt, tag="out")
                    if dma_only:
                        nc.vector.tensor_copy(ot[:], bt[:])
                        store_eng.dma_start(out[srow : srow + S_BLK, :], ot[:])
                        continue
                    # up-projection (K=32) into wide PSUM chunks + base add
                    for off, width in CHUNK_PLAN:
                        pd = pd_pool.tile([S_BLK, 1536], f32, tag="pd")
                        for i in range(width // 512):
                            nc.tensor.matmul(
                                pd[:, i * 512 : (i + 1) * 512],
                                lowT[:, g * S_BLK : (g + 1) * S_BLK],
                                bwt_sb[:, off + i * 512 : off + (i + 1) * 512],
                                start=True,
                                stop=True,
                            )
                        nc.vector.tensor_add(
                            ot[:, off : off + width],
                            pd[:, :width],
                            bt[:, off : off + width],
                        )
                    store_eng.dma_start(out[srow : srow + S_BLK, :], ot[:])

    _split_sync_waits(nc)
    return nc


def make_in_maps(x, base_output, lora_A, lora_B, top_k_weights, top_k_indices):
    """Host-side prep: expert gather, gate/scaling fold, x h-major relayout,
    compression to device I/O dtypes, and (i8 mode) output-scale calibration.
    Stores the dequantization scale in _CACHE['out_scale']."""
    import concourse.mybir as mybir

    xt_dt, tab_dt, base_dt, out_dt = _dtypes()
    np_xt = mybir.dt.np(xt_dt)
    np_tab = mybir.dt.np(tab_dt)
    np_base = mybir.dt.np(base_dt)

    x = np.asarray(x, dtype=np.float32)
    base_output = np.asarray(base_output, dtype=np.float32)
    lora_A = np.asarray(lora_A, dtype=np.float32)
    lora_B = np.asarray(lora_B, dtype=np.float32)
    w = np.asarray(top_k_weights, dtype=np.float32)
    idx = np.asarray(top_k_indices)

    A_sel = lora_A[idx]  # [B, K, R, H]
    At = A_sel.reshape(B, KR, H).transpose(0, 2, 1)  # [B, H, 32]
    # stripe h-major: At_dev[b, p, j, r] = At[b, j*128 + p, r]
    At_dev = np.ascontiguousarray(
        At.reshape(B, HB, 128, KR).transpose(0, 2, 1, 3)
    ).astype(np_tab)  # [B, 128, 28, 32]
    B_sel = lora_B[idx]  # [B, K, H, R]
    Bw = B_sel * (w * SCALING)[:, :, None, None]
    Bwt = np.ascontiguousarray(
        Bw.transpose(0, 1, 3, 2).reshape(B, KR, H)
    )  # [B, 32, H]

    if OUT_MODE == "i8":
        # Calibrate the int8 output scale from an exact delta on a 1/64 row
        # sample; absmax of the full tensor exceeds the sample absmax only
        # by the Gaussian-extreme ratio (~1.15x), covered by SCALE_MARGIN.
        xs = x[:, ::64].astype(np.float32)  # [B, 32, H]
        low_s = np.einsum("bsh,bhr->bsr", xs, At)
        delta_s = np.einsum("bsr,brh->bsh", low_s, Bwt)
        samp_max = float(np.abs(delta_s + base_output[:, ::64]).max())
        scale = max(samp_max, 1e-6) * SCALE_MARGIN / 127.0
    else:
        scale = 1.0
    _CACHE["out_scale"] = scale

    Bwt_dev = (Bwt / scale).astype(np_tab)
    base_dev = (base_output / scale).astype(np_base)

    # x -> xt[half, p, j, s]: partition-major tiles, each half fully
    # contiguous per SBUF partition line
    # xt[b, 2m+hf, p, j, s] = x[b, m*512 + s, hf*1792 + j*128 + p]
    xt = np.ascontiguousarray(
        x.astype(np_xt)
        .reshape(B, NMAC, 512, 2, HHALF, 128)
        .transpose(0, 1, 3, 5, 4, 2)  # [B, m, hf, p, j, s]
        .reshape(B, 2 * NMAC, 128, HHALF, 512)
    )

    return [
        {
            "xt": xt[b],
            "base": np.ascontiguousarray(base_dev[b]),
            "at": At_dev[b],
            "bwt": Bwt_dev[b],
        }
        for b in range(B)
    ]


def kernel(x, base_output, lora_A, lora_B, top_k_weights, top_k_indices):
    from concourse.bass_utils import run_bass_kernel_spmd

    nc = _CACHE.get("nc")
    if nc is None:
        nc = build_nc()
        _CACHE["nc"] = nc

    in_maps = make_in_maps(
        x, base_output, lora_A, lora_B, top_k_weights, top_k_indices
    )
    scale = _CACHE["out_scale"]
    res = run_bass_kernel_spmd(nc, in_maps, list(range(N_CORES)))
    out = np.stack(
        [np.asarray(res.results[b]["out"]) for b in range(B)], axis=0
    ).astype(np.float32)
    if scale != 1.0:
        out *= scale
    return out


# revision 20
# speedup vs baseline: 1.1392x; 1.0031x over previous
"""Expert-LoRA routed delta kernel for Trainium2 (8 NeuronCores).

Math (per batch b, with routing resolved on host):
    out[b] = base[b] + x[b] @ At_b @ Bwt_b
where
    At_b  [H, 32] = concat_k A_{e_k}^T              (e_k = top_k_indices[b, k])
    Bwt_b [32, H] = concat_k (w_{b,k} * scaling * B_{e_k}^T)

Host-side prep folds everything cheap into input layout:
  * expert gather + gate weights + lora scaling -> tiny At/Bwt tables;
  * x is pre-transposed to an h-major tiled layout (xt[half, p, j, s]) so the
    tensor engine can contract over H without any on-chip transposes, and
    each DMA reads one fully contiguous block.

The kernel is HBM-bandwidth-bound (~358 GB/s per NeuronCore), so all bulk
I/O is compressed (rel-err budget 2e-2; measured total err ~1.1e-2 worst):
  * x in bf16 (14.7 MB),
  * base in fp8, pre-divided by a host-calibrated output scale (7.3 MB),
  * out in int8 in the scaled domain (7.3 MB), dequantized on host.
The scale folds into Bwt (bwt_s = Bwt/scale) so the device program is
scale-free. Per-core traffic: 88 MB -> 29.4 MB.

Device pipeline per core (= one batch; B == n_cores == 8):
  for each 512-row S-macro: load xT halves + whole-macro base tile (I/O
  batched to ~1.8 MB DMAs — measured ~14% faster than 459 KB-granular
  I/O) -> 28 accumulating matmuls (rank-32 down-projection, N=512) ->
  per 128-row block: 7 up-projection matmuls (K=32, N=512) grouped into
  1536-wide PSUM chunks + wide vector add with base (wide chunks
  amortize the DVE ~151-cycle/op overhead) -> whole-macro store.
  DVE ~65us, PE ~48us busy; measured 87us/core ~= the shared-HBM
  roofline (29.4 MB at ~340 GB/s/core effective).

Sharding: data-parallel over batch (spec sharding_hint), SPMD program.
"""

import sys

if "/opt/trn_rl_repo" not in sys.path:
    sys.path.insert(0, "/opt/trn_rl_repo")

import numpy as np

# Problem shape (hardcoded per contract; must match setup_inputs()).
B, S, H = 8, 2048, 3584
E, R, TOPK = 8, 16, 2
KR = TOPK * R  # 32 = concatenated rank
SCALING = 32.0 / 16.0
N_CORES = 8

S_BLK = 128
NS = S // S_BLK  # 16 s-blocks
HB = H // 128  # 28 h-blocks of 128
HC = H // 512  # 7 h-chunks of 512
NMAC = S // 512  # 4 S-macros of 512 rows
HHALF = HB // 2  # 14 h-blocks per xT half-tile
CHUNK_PLAN = [(0, 1536), (1536, 1536), (3072, 512)]  # up-proj (h_off, width)

# I/O compression config.
XT_DT = "bfloat16"
TAB_DT = "bfloat16"  # at / bwt tables (tiny but must match matmul operand)
OUT_MODE = "i8"  # "i8": scaled int8 out + fp8 scaled base | "bf16"
SCALE_MARGIN = 1.30  # quantization headroom over sampled absmax
X_MERGE = True  # whole-macro x tiles (must match build_nc(x_merge=...))

_CACHE: dict = {}


def _dtypes():
    import concourse.mybir as mybir

    if OUT_MODE == "i8":
        # base is pre-divided by scale (values ~<0.2): e4m3 keeps them in
        # normal range; even flushed subnormals would be harmless.
        return getattr(mybir.dt, XT_DT), getattr(mybir.dt, TAB_DT), \
            mybir.dt.float8e4, mybir.dt.int8
    return getattr(mybir.dt, XT_DT), getattr(mybir.dt, TAB_DT), \
        mybir.dt.float8e3, mybir.dt.bfloat16


def _split_sync_waits(nc, max_waits=1):
    """This walrus build rejects >max_waits sync-wait commands on a single
    instruction (setupSyncWait: 'Too many sync wait commands'). Hoist excess
    waits onto same-engine NOPs inserted immediately before the instruction.
    Same-queue ordering makes this equivalent: the engine blocks on each
    hoisted wait before reaching the original instruction. Monotonic (ge)
    waits are hoisted first; eq-waits stay on the instruction when possible.
    """
    import concourse.mybir as mybir

    for fn in nc.m.functions:
        for bb in fn.blocks:
            new_insts = []
            for inst in bb.instructions:
                si = inst.sync_info
                if si is not None and si.on_wait and len(si.on_wait) > max_waits:
                    waits = list(si.on_wait)
                    ge = [w for w in waits if w.wait_mode != "sem-eq-imm"]
                    eq = [w for w in waits if w.wait_mode == "sem-eq-imm"]
                    keep = (eq + ge)[-max_waits:]
                    hoist = (eq + ge)[:-max_waits]
                    for w in hoist:
                        new_insts.append(
                            mybir.InstNoOp(
                                name=f"I-{nc.next_id()}",
                                engine=inst.engine,
                                bass_nofuse=True,
                                sync_info=mybir.SyncInfo(on_wait=[w], on_update=[]),
                            )
                        )
                    inst.sync_info = mybir.SyncInfo(
                        on_wait=keep, on_update=list(si.on_update or [])
                    )
                new_insts.append(inst)
            bb.instructions[:] = new_insts


def build_nc(reps=1, dma_only=False, io_bufs=2, xt_bufs=None,
             store_on_act=True, base_eng="sync", batch_io=True,
             x_merge=True):
    """Build the single-core Bass program (SPMD: same program on all cores).

    reps>1 repeats the whole pipeline (same I/O, idempotent) — used only for
    slope-based device-time measurement in test.py. dma_only strips compute
    (out <- cast(base), xT still loaded) to calibrate the pure DMA roofline.
    """
    import concourse.bass as bass
    import concourse.mybir as mybir
    import concourse.tile as tile

    f32 = mybir.dt.float32
    xt_dt, tab_dt, base_dt, out_dt = _dtypes()

    if xt_bufs is None:
        xt_bufs = 2 if x_merge else 4

    nc = bass.Bass()
    # partition-major x relayout so each tile loads as one fully contiguous
    # DMA; x_merge: one whole-macro tile, else two half-tiles
    # merged:  xt[m, p, j, s]     = x[m*512 + s, j*128 + p]
    # halves:  xt[2m+hf, p, j, s] = x[m*512 + s, hf*1792 + j*128 + p]
    assert x_merge == X_MERGE, "host xt layout must match the device program"
    if x_merge:
        xt = nc.dram_tensor("xt", [NMAC, 128, HB, 512], xt_dt, kind="ExternalInput")
    else:
        xt = nc.dram_tensor(
            "xt", [2 * NMAC, 128, HHALF, 512], xt_dt, kind="ExternalInput"
        )
    base = nc.dram_tensor("base", [S, H], base_dt, kind="ExternalInput")
    # at[p, j, r] = A_cat^T[j*128 + p, r] (pre-striped on host)
    at = nc.dram_tensor("at", [128, HB, KR], tab_dt, kind="ExternalInput")
    bwt = nc.dram_tensor("bwt", [KR, H], tab_dt, kind="ExternalInput")
    out = nc.dram_tensor("out", [S, H], out_dt, kind="ExternalOutput")

    # Loads go on the SP HWDGE ring; stores on the ACT ring so a store
    # waiting for compute never head-of-line-blocks the next loads.
    store_eng = nc.scalar if store_on_act else nc.sync
    b_eng = {"sync": nc.sync, "scalar": nc.scalar, "gpsimd": nc.gpsimd}[base_eng]

    with tile.TileContext(nc) as tc:
        with (
            tc.tile_pool(name="const", bufs=1) as const_pool,
            tc.tile_pool(name="xth", bufs=xt_bufs) as xt_pool,
            tc.tile_pool(name="bin", bufs=io_bufs) as b_pool,
            tc.tile_pool(name="oout", bufs=io_bufs) as o_pool,
            tc.tile_pool(name="low", bufs=3) as low_pool,
            tc.tile_pool(name="plow", bufs=2, space="PSUM") as plow_pool,
            tc.tile_pool(name="pd", bufs=2, space="PSUM") as pd_pool,
        ):
            # Const tables ride the (idle-at-start) store ring so they land
            # concurrently with the first xT load on the SP ring.
            at_sb = const_pool.tile([128, HB, KR], tab_dt)
            store_eng.dma_start(at_sb[:], at[:])
            bwt_sb = const_pool.tile([KR, H], tab_dt)
            store_eng.dma_start(bwt_sb[:], bwt[:])

            for m in range(NMAC * reps):
                m = m % NMAC
                # xT tiles: [128 h-partitions, h-blocks, 512 s]
                if x_merge:
                    xf = xt_pool.tile([128, HB, 512], xt_dt, tag="xth")
                    nc.sync.dma_start(xf[:], xt[m])
                    xop = lambda j: xf[:, j, :]
                else:
                    halves = []
                    for hf in range(2):
                        xh = xt_pool.tile([128, HHALF, 512], xt_dt, tag="xth")
                        nc.sync.dma_start(xh[:], xt[2 * m + hf])
                        halves.append(xh)
                    xop = lambda j: halves[j // HHALF][:, j % HHALF, :]

                if not dma_only:
                    # down-projection: lowT[kr, s] = sum_h At[h, kr] * xT[h, s]
                    plow = plow_pool.tile([KR, 512], f32, tag="plow")
                    for j in range(HB):
                        nc.tensor.matmul(
                            plow[:],
                            at_sb[:, j, :],
                            xop(j),
                            start=(j == 0),
                            stop=(j == HB - 1),
                        )
                    lowT = low_pool.tile([KR, 512], tab_dt, tag="lowT")
                    nc.vector.tensor_copy(lowT[:], plow[:])

                if batch_io:
                    # whole-macro base load / out store: one 512-row DMA
                    # each, laid out [p, k, h] (k = s-block within macro)
                    mrows = slice(m * 512, (m + 1) * 512)
                    bt4 = b_pool.tile([S_BLK, 4, H], base_dt, tag="base")
                    b_eng.dma_start(
                        bt4[:], base[mrows].rearrange("(k p) h -> p k h", p=128)
                    )
                    ot4 = o_pool.tile([S_BLK, 4, H], out_dt, tag="out")
                    for g in range(4):
                        if dma_only:
                            nc.vector.tensor_copy(ot4[:, g, :], bt4[:, g, :])
                            continue
                        for off, width in CHUNK_PLAN:
                            pd = pd_pool.tile([S_BLK, 1536], f32, tag="pd")
                            for i in range(width // 512):
                                nc.tensor.matmul(
                                    pd[:, i * 512 : (i + 1) * 512],
                                    lowT[:, g * S_BLK : (g + 1) * S_BLK],
                                    bwt_sb[:, off + i * 512 : off + (i + 1) * 512],
                                    start=True,
                                    stop=True,
                                )
                            # NOTE: adds must stay on DVE — gpsimd has no
                            # PSUM port (attempting it wedges the exec unit)
                            nc.vector.tensor_add(
                                ot4[:, g, off : off + width],
                                pd[:, :width],
                                bt4[:, g, off : off + width],
                            )
                    store_eng.dma_start(
                        out[mrows].rearrange("(k p) h -> p k h", p=128), ot4[:]
                    )
                    continue

                for g in range(4):  # 128-row s-blocks within the macro
                    srow = m * 512 + g * S_BLK
                    bt = b_pool.tile([S_BLK, H], base_dt, tag="base")
                    b_eng.dma_start(bt[:], base[srow : srow + S_BLK, :])
                    ot = o_pool.tile([S_BLK, H], out_dt, tag="out")
                    if dma_only:
                        nc.vector.tensor_copy(ot[:], bt[:])
                        store_eng.dma_start(out[srow : srow + S_BLK, :], ot[:])
                        continue
                    # up-projection (K=32) into wide PSUM chunks + base add
                    for off, width in CHUNK_PLAN:
                        pd = pd_pool.tile([S_BLK, 1536], f32, tag="pd")
                        for i in range(width // 512):
                            nc.tensor.matmul(
                                pd[:, i * 512 : (i + 1) * 512],
                                lowT[:, g * S_BLK : (g + 1) * S_BLK],
                                bwt_sb[:, off + i * 512 : off + (i + 1) * 512],
                                start=True,
                                stop=True,
                            )
                        nc.vector.tensor_add(
                            ot[:, off : off + width],
                            pd[:, :width],
                            bt[:, off : off + width],
                        )
                    store_eng.dma_start(out[srow : srow + S_BLK, :], ot[:])

    _split_sync_waits(nc)
    return nc


def make_in_maps(x, base_output, lora_A, lora_B, top_k_weights, top_k_indices):
    """Host-side prep: expert gather, gate/scaling fold, x h-major relayout,
    compression to device I/O dtypes, and (i8 mode) output-scale calibration.
    Stores the dequantization scale in _CACHE['out_scale']."""
    import concourse.mybir as mybir

    xt_dt, tab_dt, base_dt, out_dt = _dtypes()
    np_xt = mybir.dt.np(xt_dt)
    np_tab = mybir.dt.np(tab_dt)
    np_base = mybir.dt.np(base_dt)

    x = np.asarray(x, dtype=np.float32)
    base_output = np.asarray(base_output, dtype=np.float32)
    lora_A = np.asarray(lora_A, dtype=np.float32)
    lora_B = np.asarray(lora_B, dtype=np.float32)
    w = np.asarray(top_k_weights, dtype=np.float32)
    idx = np.asarray(top_k_indices)

    A_sel = lora_A[idx]  # [B, K, R, H]
    At = A_sel.reshape(B, KR, H).transpose(0, 2, 1)  # [B, H, 32]
    # stripe h-major: At_dev[b, p, j, r] = At[b, j*128 + p, r]
    At_dev = np.ascontiguousarray(
        At.reshape(B, HB, 128, KR).transpose(0, 2, 1, 3)
    ).astype(np_tab)  # [B, 128, 28, 32]
    B_sel = lora_B[idx]  # [B, K, H, R]
    Bw = B_sel * (w * SCALING)[:, :, None, None]
    Bwt = np.ascontiguousarray(
        Bw.transpose(0, 1, 3, 2).reshape(B, KR, H)
    )  # [B, 32, H]

    if OUT_MODE == "i8":
        # Calibrate the int8 output scale from an exact delta on a 1/64 row
        # sample; absmax of the full tensor exceeds the sample absmax only
        # by the Gaussian-extreme ratio (~1.15x), covered by SCALE_MARGIN.
        xs = x[:, ::64].astype(np.float32)  # [B, 32, H]
        low_s = np.einsum("bsh,bhr->bsr", xs, At)
        delta_s = np.einsum("bsr,brh->bsh", low_s, Bwt)
        samp_max = float(np.abs(delta_s + base_output[:, ::64]).max())
        scale = max(samp_max, 1e-6) * SCALE_MARGIN / 127.0
    else:
        scale = 1.0
    _CACHE["out_scale"] = scale

    Bwt_dev = (Bwt / scale).astype(np_tab)
    base_dev = (base_output / scale).astype(np_base)

    # x -> partition-major tiles, contiguous per SBUF partition line
    if X_MERGE:
        # xt[b, m, p, j, s] = x[b, m*512 + s, j*128 + p]
        xt = np.ascontiguousarray(
            x.astype(np_xt)
            .reshape(B, NMAC, 512, HB, 128)
            .transpose(0, 1, 4, 3, 2)  # [B, m, p, j, s]
        )
    else:
        # xt[b, 2m+hf, p, j, s] = x[b, m*512 + s, hf*1792 + j*128 + p]
        xt = np.ascontiguousarray(
            x.astype(np_xt)
            .reshape(B, NMAC, 512, 2, HHALF, 128)
            .transpose(0, 1, 3, 5, 4, 2)  # [B, m, hf, p, j, s]
            .reshape(B, 2 * NMAC, 128, HHALF, 512)
        )

    return [
        {
            "xt": xt[b],
            "base": np.ascontiguousarray(base_dev[b]),
            "at": At_dev[b],
            "bwt": Bwt_dev[b],
        }
        for b in range(B)
    ]


def kernel(x, base_output, lora_A, lora_B, top_k_weights, top_k_indices):
    from concourse.bass_utils import run_bass_kernel_spmd

    nc = _CACHE.get("nc")
    if nc is None:
        nc = build_nc()
        _CACHE["nc"] = nc

    in_maps = make_in_maps(
        x, base_output, lora_A, lora_B, top_k_weights, top_k_indices
    )
    scale = _CACHE["out_scale"]
    res = run_bass_kernel_spmd(nc, in_maps, list(range(N_CORES)))
    out = np.stack(
        [np.asarray(res.results[b]["out"]) for b in range(B)], axis=0
    ).astype(np.float32)
    if scale != 1.0:
        out *= scale
    return out
